# revision 8
# baseline (speedup 1.0000x reference)
"""L1-attention scores on 8 Trainium2 NeuronCores.

out[b, i, j, h] = -1/sqrt(w) * sum_w |q[b, j, h, w] - k[b, i, h, w]|
  q, k: [2, 2048, 8, 64] f32  ->  out: [2, 2048, 2048, 8] f32

Sharding: the 16 (b, h) pairs are split 2-per-core across 8 cores (SPMD,
no collectives). Per pair, SBUF partitions carry (i_local in 0..31,
w_sub in 0..3): DVE/ACT produce |q[j,w] - k[i,w]| tiles [128, 2048]
(bf16, one per w-chunk of 4) and the TensorEngine contracts them with a
constant [128, 32] selector whose entries are -1/8 (folds the output
scale), accumulating over the 16 w-chunks into [32, 512] PSUM quadrants.
"""

import numpy as np

BS, N, H, W = 2, 2048, 8, 64
NCORES = 8
PAIRS_PER_CORE = (BS * H) // NCORES  # 2
SCALE = -1.0 / 8.0  # -1/sqrt(64), exact in bf16
IL = 32              # i rows per matmul (PE col-quadrant)
WC = 4               # w's per chunk; IL * WC = 128 partitions
NWC = W // WC        # 16 w-chunks
NIG = N // IL        # 64 i-groups per head
JQ = 512             # matmul free-dim (one PSUM bank)
NJQ = N // JQ        # 4

_cached = None


def _build(loop_n=None):
    """Build the Bass module. loop_n wraps the body in a hardware For_i
    repeating it loop_n times (timing builds only)."""
    import contextlib

    import concourse.bass as bass
    import concourse.tile as tile
    from concourse import bacc, mybir

    nc = bacc.Bacc("TRN2", target_bir_lowering=False, debug=False,
                   num_devices=NCORES)

    qb_d = nc.dram_tensor("qb", [PAIRS_PER_CORE, NWC, 128, N],
                          mybir.dt.bfloat16, kind="ExternalInput")
    kt_d = nc.dram_tensor("kt", [PAIRS_PER_CORE, 128, NIG * NWC],
                          mybir.dt.float32, kind="ExternalInput")
    sneg_d = nc.dram_tensor("sneg", [128, IL], mybir.dt.bfloat16,
                            kind="ExternalInput")
    out_d = nc.dram_tensor("out", [PAIRS_PER_CORE, N, N], mybir.dt.float32,
                           kind="ExternalOutput")

    with tile.TileContext(nc) as tc:
        with (
            tc.tile_pool(name="const", bufs=1) as constp,
            tc.tile_pool(name="qbp", bufs=NWC + 2) as qbp,
            tc.tile_pool(name="absp", bufs=6) as absp,
            tc.tile_pool(name="outp", bufs=3) as outp,
            tc.tile_pool(name="psump", bufs=8, space=bass.MemorySpace.PSUM) as psump,
        ):
            sneg_sb = constp.tile([128, IL], mybir.dt.bfloat16, tag="sneg")
            nc.sync.dma_start(sneg_sb[:], sneg_d.ap())

            loop_cm = (tc.For_i(0, loop_n, 1) if loop_n
                       else contextlib.nullcontext())
            with loop_cm:
                _emit_body(nc, tc, mybir, constp, qbp, absp, outp, psump,
                           sneg_sb, qb_d, kt_d, out_d)

    nc.compile()
    return nc


def _emit_body(nc, tc, mybir, constp, qbp, absp, outp, psump,
               sneg_sb, qb_d, kt_d, out_d):
    if True:
            for p in range(PAIRS_PER_CORE):
                kt_sb = constp.tile([128, NIG * NWC], mybir.dt.float32,
                                    tag=f"kt{p}")
                ktn_sb = constp.tile([128, NIG * NWC], mybir.dt.float32,
                                     tag=f"ktn{p}")
                nc.sync.dma_start(kt_sb[:], kt_d.ap()[p])
                # negated k for the ACT bias path: |x + (-k)|
                nc.vector.tensor_scalar(ktn_sb[:], kt_sb[:], -1.0, None,
                                        mybir.AluOpType.mult)
                qb_sb = []
                for wc in range(NWC):
                    t = qbp.tile([128, N], mybir.dt.bfloat16, tag="qb")
                    nc.sync.dma_start(t[:], qb_d.ap()[p, wc])
                    qb_sb.append(t)

                for ig in range(NIG):
                    ig4 = ig % 4
                    if ig4 == 0:
                        ps = [psump.tile([128, JQ], mybir.dt.float32,
                                         tag="ps", name=f"ps{jq}")
                              for jq in range(NJQ)]
                    for wc in range(NWC):
                        col = ig * NWC + wc
                        a_t = absp.tile([128, N], mybir.dt.bfloat16, tag="a")
                        if col % 13 < 8:
                            # DVE: d = qb - k (4x), then clear sign bit (4x)
                            d_t = absp.tile([128, N], mybir.dt.bfloat16,
                                            tag="d")
                            nc.vector.tensor_scalar(
                                d_t[:], qb_sb[wc][:],
                                kt_sb[:, col:col + 1], None,
                                mybir.AluOpType.subtract)
                            nc.vector.tensor_scalar(
                                a_t[:].bitcast(mybir.dt.uint16),
                                d_t[:].bitcast(mybir.dt.uint16),
                                0x7FFF, None, mybir.AluOpType.bitwise_and)
                        else:
                            # ACT: |1*qb + (-k)|
                            nc.scalar.activation(
                                a_t[:], qb_sb[wc][:],
                                mybir.ActivationFunctionType.Abs,
                                bias=ktn_sb[:, col:col + 1], scale=1.0)
                        for jq in range(NJQ):
                            nc.tensor.matmul(
                                ps[jq][IL * ig4:IL * (ig4 + 1), :],
                                sneg_sb[:],
                                a_t[:, jq * JQ:(jq + 1) * JQ],
                                start=(wc == 0), stop=(wc == NWC - 1),
                                tile_position=(0, IL * ig4))
                    if ig4 == 3:
                        o_t = outp.tile([128, N], mybir.dt.float32, tag="o")
                        for jq in range(NJQ):
                            dst = o_t[:, jq * JQ:(jq + 1) * JQ]
                            if jq % 2 == 0:
                                nc.scalar.copy(dst, ps[jq][:])
                            else:
                                nc.vector.tensor_copy(dst, ps[jq][:])
                        r0 = (ig - 3) * IL
                        nc.sync.dma_start(
                            out_d.ap()[p, r0:r0 + 128, :], o_t[:])


def _prep_inputs(q, k):
    import ml_dtypes

    qf = np.ascontiguousarray(q, dtype=np.float32)
    kf = np.ascontiguousarray(k, dtype=np.float32)
    sneg = np.zeros((128, IL), dtype=ml_dtypes.bfloat16)
    for il in range(IL):
        sneg[il * WC:(il + 1) * WC, il] = SCALE
    in_maps = []
    for c in range(NCORES):
        qb = np.empty((PAIRS_PER_CORE, NWC, 128, N), dtype=ml_dtypes.bfloat16)
        kt = np.empty((PAIRS_PER_CORE, 128, NIG * NWC), dtype=np.float32)
        for p in range(PAIRS_PER_CORE):
            g = c * PAIRS_PER_CORE + p
            b, hd = g // H, g % H
            qt = qf[b, :, hd, :].T.astype(ml_dtypes.bfloat16)  # [w, j]
            for wc in range(NWC):
                qb[p, wc] = np.tile(qt[WC * wc:WC * (wc + 1)], (IL, 1))
            # kt[(i_l, w_sub), ig*NWC + wc] = k[b, IL*ig + i_l, hd, WC*wc + w_sub]
            kt[p] = (kf[b, :, hd, :]
                     .reshape(NIG, IL, NWC, WC).transpose(1, 3, 0, 2)
                     .reshape(128, NIG * NWC))
        in_maps.append({"qb": qb, "kt": kt, "sneg": sneg})
    return in_maps


def kernel(q, k):
    from concourse.bass_utils import run_bass_kernel_spmd

    global _cached
    if _cached is None:
        _cached = _build()
    nc = _cached

    in_maps = _prep_inputs(q, k)
    res = run_bass_kernel_spmd(nc, in_maps, core_ids=list(range(NCORES)))

    out = np.empty((BS, N, N, H), dtype=np.float32)
    for c in range(NCORES):
        o = res.results[c]["out"]
        for p in range(PAIRS_PER_CORE):
            g = c * PAIRS_PER_CORE + p
            b, hd = g // H, g % H
            out[b, :, :, hd] = o[p]
    return out
